# revision 1
# baseline (speedup 1.0000x reference)
"""GAT local-attention kernel, data-parallel over 8 NeuronCores.

Shapes (hardcoded per problem spec):
  neibor_embedding [4, 1024, 32, 512] f32
  mask             [4, 1024, 32]      i32
  x                [4, 1024, 512]     f32
  Wq/Wk/Wv         [512, 512] f32, bq/bk/bv [512] f32
Output: [4, 1024, 512] f32

Sharding: flatten (B, N) -> 4096 tokens, split 512 tokens per core.
Attention is strictly local per token neighborhood, so no cross-core
communication is needed; Linear weights are replicated.
"""
import numpy as np
import jax
import jax.numpy as jnp
from functools import partial

B, N, K, D = 4, 1024, 32, 512
H = 4
DH = D // H
NEG = -1.0e9
NCORES = 8


def _attn_shard(neib, mask, x, Wq, bq, Wk, bk, Wv, bv):
    # neib [T, K, D], mask [T, K], x [T, D]  (T = tokens in shard)
    T = x.shape[0]
    q = x @ Wq.T + bq                                  # [T, D]
    k = neib @ Wk.T + bk                               # [T, K, D]
    v = neib @ Wv.T + bv                               # [T, K, D]
    qh = q.reshape(T, H, DH)
    kh = k.reshape(T, K, H, DH)
    vh = v.reshape(T, K, H, DH)
    scores = jnp.einsum('thd,tkhd->thk', qh, kh)       # [T, H, K]
    m = (mask[:, None, :] == 0)                        # [T, 1, K]
    scores = jnp.where(m, NEG, scores)
    attn = jax.nn.softmax(scores, axis=-1)
    vec = jnp.einsum('thk,tkhd->thd', attn, vh)        # [T, H, DH]
    return vec.reshape(T, D)


def kernel(neibor_embedding, mask, x, Wq, bq, Wk, bk, Wv, bv):
    neib = np.ascontiguousarray(neibor_embedding, dtype=np.float32)
    msk = np.ascontiguousarray(mask)
    xx = np.ascontiguousarray(x, dtype=np.float32)

    devs = jax.devices()[:NCORES]
    ncores = len(devs) if len(devs) >= 1 else 1
    tot = B * N
    tpc = tot // ncores  # tokens per core

    neib_s = neib.reshape(ncores, tpc, K, D)
    mask_s = msk.reshape(ncores, tpc, K)
    x_s = xx.reshape(ncores, tpc, D)

    fn = jax.pmap(
        _attn_shard,
        in_axes=(0, 0, 0, None, None, None, None, None, None),
        devices=devs,
    )
    out = fn(neib_s, mask_s, x_s,
             jnp.asarray(Wq), jnp.asarray(bq),
             jnp.asarray(Wk), jnp.asarray(bk),
             jnp.asarray(Wv), jnp.asarray(bv))
    out = np.asarray(out).reshape(B, N, D).astype(np.float32)
    return out

